# revision 31
# baseline (speedup 1.0000x reference)
"""Trainium2 Bass kernel for nn_MultiHeadAttention_63986422775834.

Computation (see harness reference):
    q = x @ Wq + bq; k = x @ Wk + bk; v = x @ Wv + bv          # [N, D]
    group rows by 8: scores[b,h,g] = q[8b+h] . k[8b+g] / sqrt(D)
    w = softmax(scores, axis=-1);  out[8b+h] = sum_g w[b,h,g] * v[8b+g]

Algebraic restructuring: scores[i,j] = x_i (Wq Wk^T) x_j^T + const_i
+ x_j.(Wk bq) + const, and the const_i terms drop under softmax; so with
A = Wq Wk^T (shared by ALL rows on ALL cores) and beta = Wk bq:
    scores[i,j] = u_i . x_j + b_j,   u = x @ A,   b = x @ beta
This replaces the two q/k projection GEMMs (2 x 2048^3 MACs/core) with
ONE GEMM u = x @ A plus a SHARDED A: each core computes a 256-row slice
of A and 4 column-group AllGathers share it (~7us each, hidden).  bv
rides through softmax (rows sum to 1) folded into the V tiles; b rides
as an extra appended column on the V c0 chunk and enters each score
PSUM chain as a rank-1 matmul.

Schedule (the PE clock governor rewards an always-busy tensor engine,
so the kernel is ordered to keep the PE stream gap-free from ~10us):
  phase A: x at full DMA priority -> xT transposes + V c0 (129 wide,
           with the beta column) keep the PE fed while Wk streams.
  g-loop:  per 512-row quarter of Wk: transposes -> A-slice chunk ->
           AllGather_g; u-chunks of quarter g-1 run between, so no PE
           idle while a collective is in flight.  S partials of a-chunk
           n interleave into the u-chains of chunk n+1.
  tail:    last S partials + rank-1 b + softmax + O c0, then V c1..c4
           (512-wide) with O, all dense.
Stores (A bounce, outputs) ride the scalar-engine DMA ring so they
cannot block the load stream on the sync ring.
"""

import sys

sys.path.insert(0, "/opt/trn_rl_repo")

import numpy as np
import ml_dtypes

import concourse.mybir as mybir
import concourse.tile as tile
from concourse import bacc
from concourse.bass_utils import run_bass_kernel_spmd

# problem shape (hardcoded per contract)
N_FULL = 16384
D = 2048
H = 8
N_CORES = 8
R = N_FULL // N_CORES  # rows per core = 2048
P = 128
KO = D // P  # 16 k-subtiles along d_in
SL = D // N_CORES  # A rows computed per core = 256
SCALE = 1.0 / np.sqrt(np.float32(D))

BF16 = mybir.dt.bfloat16
F32 = mybir.dt.float32

BLOCKS = [(0, 512), (512, 512), (1024, 512), (1536, 512)]
N_SUB = R // P  # 16 row subtiles
MC = 4  # Wk quarters == A column groups == AllGathers
ACH = 8  # gathered-A chunks of 256 cols (2 m-tiles each)
V_W0 = 128  # V c0 real width (col 128 carries b = x.beta)
V_CHUNKS = [(128, 512), (640, 512), (1152, 512), (1664, 384)]


def build_program():
    nc = bacc.Bacc("TRN2", target_bir_lowering=False, debug=False, num_devices=N_CORES)

    xs = nc.dram_tensor("xs", [R, D], F32, kind="ExternalInput")
    Wk = nc.dram_tensor("Wk", [D, D], F32, kind="ExternalInput")
    Wv = nc.dram_tensor("Wv", [D, D], F32, kind="ExternalInput")
    wq_sl = nc.dram_tensor("wq_sl", [SL, D], F32, kind="ExternalInput")
    beta_col = nc.dram_tensor("beta_col", [P, KO], F32, kind="ExternalInput")
    bvb = nc.dram_tensor("bvb", [P, D], BF16, kind="ExternalInput")
    maskt = nc.dram_tensor("maskt", [P, P], F32, kind="ExternalInput")
    ident = nc.dram_tensor("ident", [P, P], BF16, kind="ExternalInput")
    ones1 = nc.dram_tensor("ones1", [1, P], BF16, kind="ExternalInput")
    out = nc.dram_tensor("out", [R, D], F32, kind="ExternalOutput")

    wv_ap = Wv[:].rearrange("(ko p) n -> p ko n", p=P)

    from contextlib import ExitStack

    with tile.TileContext(nc) as tc:
        with ExitStack() as stack:
            pool = lambda *a, **kw: stack.enter_context(tc.tile_pool(*a, **kw))
            const = pool(name="const", bufs=1)
            xT_pool = pool(name="xT", bufs=1)
            wkv = pool(name="wkv", bufs=2)  # wv0 -> WkT quarters -> wv chunks
            uTp = pool(name="uTp", bufs=3)  # per-a-chunk uT tiles
            apool = pool(name="achunk", bufs=2)
            wqp = pool(name="wqp", bufs=1)
            v0pool = pool(name="v0", bufs=1)
            sacc = pool(name="sacc", bufs=1)
            wtmp = pool(name="wtmp", bufs=2)
            wkbp = pool(name="wkb", bufs=2)
            phA = pool(name="phA", bufs=3)
            aoutp = pool(name="aout", bufs=2)
            vpool = pool(name="vpool", bufs=2)
            opool = pool(name="opool", bufs=2)
            soft = pool(name="soft", bufs=2)
            dram = pool(name="dram", bufs=1, space="DRAM")
            ps_big = pool(name="ps_big", bufs=3, space="PSUM")
            ps_s = pool(name="ps_s", bufs=2, space="PSUM")
            ps_t = pool(name="ps_t", bufs=2, space="PSUM")
            ps_warm = pool(name="ps_warm", bufs=1, space="PSUM")

            # --- constants (sync ring) ---
            mask_sb = const.tile([P, P], F32)
            nc.sync.dma_start(mask_sb, maskt[:])
            ident_sb = const.tile([P, P], BF16)
            nc.sync.dma_start(ident_sb, ident[:])
            ones_sb = const.tile([1, P], BF16)
            nc.sync.dma_start(ones_sb, ones1[:])
            bv_sb = const.tile([P, D], BF16)
            nc.sync.dma_start(bv_sb, bvb[:])
            beta_f = const.tile([P, KO], F32)
            nc.sync.dma_start(beta_f, beta_col[:])
            beta_bf = const.tile([P, KO], BF16)
            nc.vector.tensor_copy(beta_bf, beta_f)

            # warm-up matmuls cover the first DMA-bound microseconds
            for _ in range(60):
                wps = ps_warm.tile([P, P], F32, tag="warm", name="wps")
                nc.tensor.matmul(wps, lhsT=ident_sb, rhs=ident_sb, start=True, stop=True)

            # --- persistent SBUF intermediates ---
            xT = [
                xT_pool.tile([P, KO, nrows], BF16, name=f"xT{bi}")
                for bi, (_, nrows) in enumerate(BLOCKS)
            ]
            wqT_sl = wqp.tile([P, KO, SL], BF16, name="wqT_sl")
            S_all = sacc.tile([P, N_SUB, P], F32, name="S_all")
            wT_all = sacc.tile([P, N_SUB, P], BF16, name="wT_all")
            b_all = sacc.tile([P, N_SUB], BF16, name="b_all")
            bT_flat = sacc.tile([1, N_SUB * P], BF16, name="bT_flat")
            v0_all = v0pool.tile([P, N_SUB, V_W0], BF16, name="v0_all")

            # DRAM bounce buffers for the A AllGathers
            ag_in = [dram.tile([SL, 512], BF16, name=f"ag_in{g}") for g in range(MC)]
            ag_out = [
                dram.tile([D, 512], BF16, addr_space="Shared", name=f"ag_out{g}")
                for g in range(MC)
            ]
            ag_out_ap = [
                ag_out[g][:].rearrange("(kt p) m -> p kt m", p=P) for g in range(MC)
            ]

            # --- phase A: wq slice, wv0, x blocks (sync ring, x first) ---
            for j in range(2):
                for s in range(4):
                    xt = phA.tile([P, 512], F32, tag="xt", name="wq_f")
                    nc.sync.dma_start(
                        xt, wq_sl[j * P : (j + 1) * P, s * 512 : (s + 1) * 512]
                    )
                    xb = phA.tile([P, 512], BF16, tag="xb", name="wq_b")
                    nc.vector.tensor_copy(xb, xt)
                    for t in range(4):
                        ot = s * 4 + t
                        pst = ps_t.tile([P, P], BF16, tag="tr", name="pst")
                        nc.tensor.transpose(pst, xb[:, t * P : (t + 1) * P], ident_sb)
                        nc.vector.tensor_copy(wqT_sl[:, ot, j * P : (j + 1) * P], pst)

            # wv0: Wv cols 0:128 + beta column -> width 129 (wkv slot 0)
            wv0 = wkv.tile([P, KO, 512], BF16, tag="wkv", name="wv0")[:, :, : V_W0 + 1]
            for k0 in range(KO):
                tmp = wtmp.tile([P, 1, 512], F32, tag="wtmp", name="wv0_tmp")[
                    :, :, :V_W0
                ]
                nc.sync.dma_start(tmp, wv_ap[:, k0 : k0 + 1, 0:V_W0])
                nc.vector.tensor_copy(wv0[:, k0 : k0 + 1, 0:V_W0], tmp)
            nc.vector.tensor_copy(wv0[:, :, V_W0], beta_bf)

            def phase_a_block(bi):
                row0, nrows = BLOCKS[bi]
                for s in range(4):
                    for rt in range(nrows // P):
                        r0 = row0 + rt * P
                        xt = phA.tile([P, 512], F32, tag="xt", name="xt")
                        nc.sync.dma_start(xt, xs[r0 : r0 + P, s * 512 : (s + 1) * 512])
                        xb = phA.tile([P, 512], BF16, tag="xb", name="xb")
                        nc.vector.tensor_copy(xb, xt)
                        for t in range(4):
                            kt = s * 4 + t
                            pst = ps_t.tile([P, P], BF16, tag="tr", name="pst")
                            nc.tensor.transpose(
                                pst, xb[:, t * P : (t + 1) * P], ident_sb
                            )
                            nc.vector.tensor_copy(
                                xT[bi][:, kt, rt * P : (rt + 1) * P], pst
                            )

            def v0_block(bi):
                for rs in range(4):
                    i = bi * 4 + rs
                    psv = ps_big.tile([P, 512], F32, tag="ps_big", name="psv")[
                        :, : V_W0 + 1
                    ]
                    for kt in range(KO):
                        nc.tensor.matmul(
                            psv,
                            lhsT=xT[bi][:, kt, rs * P : (rs + 1) * P],
                            rhs=wv0[:, kt, :],
                            start=(kt == 0),
                            stop=(kt == KO - 1),
                        )
                    nc.vector.tensor_copy(b_all[:, i : i + 1], psv[:, V_W0 : V_W0 + 1])
                    nc.vector.tensor_add(
                        v0_all[:, i, :], psv[:, 0:V_W0], bv_sb[:, 0:V_W0]
                    )

            for bi in range(4):
                phase_a_block(bi)
            for bi in range(4):
                v0_block(bi)

            # b columns -> partition-0 row (rank-1 bias operand)
            for i in range(N_SUB):
                pst = ps_t.tile([P, P], BF16, tag="tr", name="pst_b")
                nc.tensor.transpose(pst[:1, :], b_all[:, i : i + 1], ident_sb)
                nc.vector.tensor_copy(bT_flat[:, i * P : (i + 1) * P], pst[:1, :])

            # --- helpers for the g-loop ---
            def wk_quarter(g, wkT_g):
                # Wk rows [512g, 512g+512) -> WkT quarter [o, ot, d2-local]
                for sl in range(4):
                    d2_0 = 512 * g + 128 * sl
                    for qq in range(4):
                        wkf = wtmp.tile([P, 1, 512], F32, tag="wtmp", name="wkf")
                        wkf2 = wkf[:].rearrange("p a b -> p (a b)")
                        nc.sync.dma_start(
                            wkf2, Wk[d2_0 : d2_0 + P, qq * 512 : (qq + 1) * 512]
                        )
                        wkb = wkbp.tile([P, 512], BF16, tag="wkb", name="wkb")
                        nc.vector.tensor_copy(wkb, wkf2)
                        for t in range(4):
                            ot = qq * 4 + t
                            pst = ps_t.tile([P, P], BF16, tag="tr", name="pst")
                            nc.tensor.transpose(
                                pst, wkb[:, t * P : (t + 1) * P], ident_sb
                            )
                            nc.vector.tensor_copy(
                                wkT_g[:, ot, 128 * sl : 128 * sl + P], pst
                            )

            def a_slice_chunk(g, wkT_g):
                for db in range(2):
                    psA = ps_big.tile([P, 512], F32, tag="ps_big", name="psA")
                    for ot in range(KO):
                        nc.tensor.matmul(
                            psA,
                            lhsT=wqT_sl[:, ot, db * P : (db + 1) * P],
                            rhs=wkT_g[:, ot, :],
                            start=(ot == 0),
                            stop=(ot == KO - 1),
                        )
                    for hh in range(2):
                        aob = aoutp.tile([P, 256], BF16, tag="aout", name="aob")
                        nc.scalar.activation(
                            aob,
                            psA[:, hh * 256 : (hh + 1) * 256],
                            mybir.ActivationFunctionType.Identity,
                        )
                        nc.scalar.dma_start(
                            ag_in[g][db * P : (db + 1) * P, hh * 256 : (hh + 1) * 256],
                            aob,
                        )
                nc.gpsimd.collective_compute(
                    "AllGather",
                    mybir.AluOpType.bypass,
                    replica_groups=[list(range(N_CORES))],
                    ins=[ag_in[g].opt()],
                    outs=[ag_out[g].opt()],
                )

            def load_a_chunk(ach):
                g, half = ach // 2, ach % 2
                dst = apool.tile([P, KO, 256], BF16, tag="ach", name="a_sb")
                nc.scalar.dma_start(
                    dst, ag_out_ap[g][:, :, half * 256 : (half + 1) * 256]
                )
                return dst

            uT = {}  # ach -> [P, 2, R] tile (uTp pool, bufs=3)

            def emit_s_pair(ach, i0):
                last = ach == ACH - 1
                for i in (i0, i0 + 1):
                    bi, rs = i // 4, i % 4
                    pss = ps_s.tile([P, P], F32, tag="pss", name="pss")
                    for ml in range(2):
                        mt = ach * 2 + ml
                        nc.tensor.matmul(
                            pss,
                            lhsT=uT[ach][:, ml, i * P : (i + 1) * P],
                            rhs=xT[bi][:, mt, rs * P : (rs + 1) * P],
                            start=(ml == 0),
                            stop=(ml == 1 and not last),
                        )
                    if last:
                        nc.tensor.matmul(
                            pss,
                            lhsT=ones_sb,
                            rhs=bT_flat[:, i * P : (i + 1) * P],
                            start=False,
                            stop=True,
                        )
                    if ach == 0:
                        nc.vector.tensor_copy(S_all[:, i, :], pss)
                    else:
                        nc.vector.tensor_add(S_all[:, i, :], S_all[:, i, :], pss)

            pending_s = None

            def u_chunk(ach, a_sb):
                nonlocal pending_s
                uT[ach] = uTp.tile([P, 2, R], BF16, tag="uT", name=f"uT{ach}")
                for j in range(8):
                    ml, bi = j // 4, j % 4
                    row0, nrows = BLOCKS[bi]
                    psu = ps_big.tile([P, 512], F32, tag="ps_big", name="psu")
                    for kt in range(KO):
                        nc.tensor.matmul(
                            psu,
                            lhsT=a_sb[:, kt, ml * P : (ml + 1) * P],
                            rhs=xT[bi][:, kt, :],
                            start=(kt == 0),
                            stop=(kt == KO - 1),
                        )
                    nc.scalar.activation(
                        uT[ach][:, ml, row0 : row0 + nrows],
                        psu,
                        mybir.ActivationFunctionType.Identity,
                    )
                    if pending_s is not None:
                        emit_s_pair(pending_s, 2 * j)
                pending_s = ach
                if pending_s - 2 in uT:
                    del uT[pending_s - 2]

            # --- g-loop: WkT quarter -> A chunk -> AllGather; u lags one g ---
            wkT_tiles = {}
            a_tiles = {}
            for g in range(MC):
                wkT_g = wkv.tile([P, KO, 512], BF16, tag="wkv", name=f"wkT{g}")
                wk_quarter(g, wkT_g)
                a_slice_chunk(g, wkT_g)
                a_tiles[2 * g] = load_a_chunk(2 * g)
                a_tiles[2 * g + 1] = load_a_chunk(2 * g + 1)
                if g > 0:
                    u_chunk(2 * (g - 1), a_tiles.pop(2 * (g - 1)))
                    u_chunk(2 * (g - 1) + 1, a_tiles.pop(2 * (g - 1) + 1))
            u_chunk(6, a_tiles.pop(6))
            u_chunk(7, a_tiles.pop(7))

            # --- tail: last S partials + softmax + O c0 ---
            def emit_softmax(i):
                tmask = soft.tile([P, P], F32, tag="tmask")
                nc.vector.tensor_add(tmask, S_all[:, i, :], mask_sb)
                e = soft.tile([P, P], F32, tag="e")
                ssum = soft.tile([P, 1], F32, tag="ssum")
                nc.scalar.activation(
                    e, tmask, mybir.ActivationFunctionType.Exp,
                    scale=float(SCALE), accum_out=ssum,
                )
                rcp = soft.tile([P, 1], F32, tag="rcp")
                nc.vector.reciprocal(rcp, ssum)
                wsb = soft.tile([P, P], BF16, tag="wsb")
                nc.vector.tensor_scalar_mul(wsb, e, rcp)
                pstw = ps_t.tile([P, P], BF16, tag="tr", name="pstw")
                nc.tensor.transpose(pstw, wsb, ident_sb)
                nc.vector.tensor_copy(wT_all[:, i, :], pstw)

            for p in range(8):
                emit_s_pair(ACH - 1, 2 * p)
                if p >= 1:
                    emit_softmax(2 * (p - 1))
                    emit_softmax(2 * (p - 1) + 1)
            pending_s = None
            emit_softmax(14)
            emit_softmax(15)
            for i in range(N_SUB):
                pso = ps_big.tile([P, 512], F32, tag="ps_big", name="pso0")[:, :V_W0]
                nc.tensor.matmul(
                    pso, lhsT=wT_all[:, i, :], rhs=v0_all[:, i, :],
                    start=True, stop=True,
                )
                o_sb = opool.tile([P, 512], F32, tag="o", name="o_sb0")[:, :V_W0]
                nc.vector.tensor_copy(o_sb, pso)
                nc.scalar.dma_start(out[i * P : (i + 1) * P, 0:V_W0], o_sb)

            # --- pass 2: V chunks c1..c4 + O ---
            def load_wv_chunk(c):
                col0, width = V_CHUNKS[c]
                dst = wkv.tile([P, KO, 512], BF16, tag="wkv", name="wv_sb")
                for k0 in range(KO):
                    tmp = wtmp.tile([P, 1, 512], F32, tag="wtmp", name="wv_tmp")[
                        :, :, :width
                    ]
                    nc.sync.dma_start(tmp, wv_ap[:, k0 : k0 + 1, col0 : col0 + width])
                    nc.vector.tensor_copy(dst[:, k0 : k0 + 1, 0:width], tmp)
                return dst

            wv_tiles = {0: load_wv_chunk(0)}
            pending_o = None

            def emit_o(v_sb, i, col0, width):
                pso = ps_big.tile([P, 512], F32, tag="ps_big", name="pso")[:, :width]
                nc.tensor.matmul(
                    pso, lhsT=wT_all[:, i, :], rhs=v_sb, start=True, stop=True
                )
                o_sb = opool.tile([P, 512], F32, tag="o", name="o_sb")[:, :width]
                nc.vector.tensor_copy(o_sb, pso)
                nc.scalar.dma_start(out[i * P : (i + 1) * P, col0 : col0 + width], o_sb)

            for c in range(len(V_CHUNKS)):
                col0, width = V_CHUNKS[c]
                if c + 1 < len(V_CHUNKS) and (c + 1) not in wv_tiles:
                    wv_tiles[c + 1] = load_wv_chunk(c + 1)
                wv_sb = wv_tiles.pop(c)
                for bi, (row0, nrows) in enumerate(BLOCKS):
                    for rs in range(4):
                        i = bi * 4 + rs
                        psv = ps_big.tile([P, 512], F32, tag="ps_big", name="psv2")[
                            :, :width
                        ]
                        for kt in range(KO):
                            nc.tensor.matmul(
                                psv,
                                lhsT=xT[bi][:, kt, rs * P : (rs + 1) * P],
                                rhs=wv_sb[:, kt, 0:width],
                                start=(kt == 0),
                                stop=(kt == KO - 1),
                            )
                        v_sb = vpool.tile([P, 512], BF16, tag="v", name="v_sb")[
                            :, :width
                        ]
                        nc.vector.tensor_add(v_sb, psv, bv_sb[:, col0 : col0 + width])
                        if pending_o is not None:
                            emit_o(*pending_o)
                        pending_o = (v_sb, i, col0, width)
            if pending_o is not None:
                emit_o(*pending_o)
                pending_o = None

    nc.compile()
    return nc


_CACHED = {}


def host_constants():
    mask = np.full((P, P), -1e9, dtype=np.float32)
    for g in range(P // H):
        mask[g * H : (g + 1) * H, g * H : (g + 1) * H] = 0.0
    identity = np.eye(P, dtype=ml_dtypes.bfloat16)
    ones_row = np.ones((1, P), dtype=ml_dtypes.bfloat16)
    return mask, identity, ones_row


def make_in_maps(x, Wq, bq, Wk, bk, Wv, bv):
    x = np.ascontiguousarray(np.asarray(x, dtype=np.float32))
    Wq = np.ascontiguousarray(np.asarray(Wq, dtype=np.float32))
    Wk = np.ascontiguousarray(np.asarray(Wk, dtype=np.float32))
    Wv = np.ascontiguousarray(np.asarray(Wv, dtype=np.float32))
    bq = np.asarray(bq, dtype=np.float32)
    bv = np.asarray(bv, dtype=np.float32)

    mask, identity, ones_row = host_constants()
    beta = Wk @ bq  # [D]; surviving score-bias term is b = x @ beta
    beta_col = np.ascontiguousarray(beta.reshape(KO, P).T)
    bvb = np.ascontiguousarray(
        np.broadcast_to(bv.astype(ml_dtypes.bfloat16), (P, D))
    )

    in_maps = []
    for i in range(N_CORES):
        in_maps.append(
            {
                "xs": x[i * R : (i + 1) * R],
                "Wk": Wk,
                "Wv": Wv,
                "wq_sl": np.ascontiguousarray(Wq[i * SL : (i + 1) * SL]),
                "beta_col": beta_col,
                "bvb": bvb,
                "maskt": mask,
                "ident": identity,
                "ones1": ones_row,
            }
        )
    return in_maps


def kernel(x, Wq, bq, Wk, bk, Wv, bv):
    if "nc" not in _CACHED:
        _CACHED["nc"] = build_program()
    nc = _CACHED["nc"]
    in_maps = make_in_maps(x, Wq, bq, Wk, bk, Wv, bv)
    res = run_bass_kernel_spmd(nc, in_maps, list(range(N_CORES)))
    return np.concatenate([res.results[i]["out"] for i in range(N_CORES)], axis=0)


# revision 44
# speedup vs baseline: 1.0236x; 1.0236x over previous
"""Trainium2 Bass kernel for nn_MultiHeadAttention_63986422775834.

Computation (see harness reference):
    q = x @ Wq + bq; k = x @ Wk + bk; v = x @ Wv + bv          # [N, D]
    group rows by 8: scores[b,h,g] = q[8b+h] . k[8b+g] / sqrt(D)
    w = softmax(scores, axis=-1);  out[8b+h] = sum_g w[b,h,g] * v[8b+g]

Algebraic restructuring: scores[i,j] = x_i (Wq Wk^T) x_j^T + const_i
+ x_j.(Wk bq) + const, and the const_i terms drop under softmax; so with
A = Wq Wk^T (shared by ALL rows on ALL cores) and beta = Wk bq:
    scores[i,j] = u_i . x_j + b_j,   u = x @ A,   b = x @ beta
This replaces the two q/k projection GEMMs (2 x 2048^3 MACs/core) with
ONE GEMM u = x @ A plus a SHARDED A: each core computes a 256-row slice
of A and 4 column-group AllGathers share it (~7us each, hidden).  bv
rides through softmax (rows sum to 1) folded into the V tiles; b rides
as an extra appended column on the V c0 chunk and enters each score
PSUM chain as a rank-1 matmul.

Schedule (the PE clock governor rewards an always-busy tensor engine,
so the kernel is ordered to keep the PE stream gap-free from ~10us):
  phase A: x at full DMA priority -> xT transposes + V c0 (129 wide,
           with the beta column) keep the PE fed while Wk streams.
  g-loop:  per 512-row quarter of Wk: transposes -> A-slice chunk ->
           AllGather_g; u-chunks of quarter g-1 run between, so no PE
           idle while a collective is in flight.  S partials of a-chunk
           n interleave into the u-chains of chunk n+1.
  tail:    last S partials + rank-1 b + softmax + O c0, then V c1..c4
           (512-wide) with O, all dense.
Stores (A bounce, outputs) ride the scalar-engine DMA ring so they
cannot block the load stream on the sync ring.
"""

import sys

sys.path.insert(0, "/opt/trn_rl_repo")

import numpy as np
import ml_dtypes

import concourse.mybir as mybir
import concourse.tile as tile
from concourse import bacc
from concourse.bass_utils import run_bass_kernel_spmd

# problem shape (hardcoded per contract)
N_FULL = 16384
D = 2048
H = 8
N_CORES = 8
R = N_FULL // N_CORES  # rows per core = 2048
P = 128
KO = D // P  # 16 k-subtiles along d_in
SL = D // N_CORES  # A rows computed per core = 256
SCALE = 1.0 / np.sqrt(np.float32(D))

BF16 = mybir.dt.bfloat16
F32 = mybir.dt.float32

BLOCKS = [(0, 512), (512, 512), (1024, 512), (1536, 512)]
N_SUB = R // P  # 16 row subtiles
MC = 4  # Wk quarters == A column groups == AllGathers
ACH = 8  # gathered-A chunks of 256 cols (2 m-tiles each)
V_W0 = 128  # V c0 real width (col 128 carries b = x.beta)
V_CHUNKS = [(128, 512), (640, 512), (1152, 512), (1664, 384)]


def build_program():
    nc = bacc.Bacc("TRN2", target_bir_lowering=False, debug=False, num_devices=N_CORES)

    xs = nc.dram_tensor("xs", [R, D], F32, kind="ExternalInput")
    Wk = nc.dram_tensor("Wk", [D, D], F32, kind="ExternalInput")
    Wv = nc.dram_tensor("Wv", [D, D], F32, kind="ExternalInput")
    wq_sl = nc.dram_tensor("wq_sl", [SL, D], F32, kind="ExternalInput")
    beta_col = nc.dram_tensor("beta_col", [P, KO], F32, kind="ExternalInput")
    bvb = nc.dram_tensor("bvb", [P, D], BF16, kind="ExternalInput")
    maskt = nc.dram_tensor("maskt", [P, P], F32, kind="ExternalInput")
    ident = nc.dram_tensor("ident", [P, P], BF16, kind="ExternalInput")
    ones1 = nc.dram_tensor("ones1", [1, P], BF16, kind="ExternalInput")
    out = nc.dram_tensor("out", [R, D], F32, kind="ExternalOutput")

    wv_ap = Wv[:].rearrange("(ko p) n -> p ko n", p=P)

    from contextlib import ExitStack

    with tile.TileContext(nc) as tc:
        with ExitStack() as stack:
            pool = lambda *a, **kw: stack.enter_context(tc.tile_pool(*a, **kw))
            const = pool(name="const", bufs=1)
            xT_pool = pool(name="xT", bufs=1)
            wkv = pool(name="wkv", bufs=2)  # WkT quarters -> wv chunks
            wv0p = pool(name="wv0p", bufs=1)
            uTp = pool(name="uTp", bufs=3)  # per-a-chunk uT tiles
            apool = pool(name="achunk", bufs=2)
            wqp = pool(name="wqp", bufs=1)
            v0pool = pool(name="v0", bufs=1)
            sacc = pool(name="sacc", bufs=1)
            wtmp = pool(name="wtmp", bufs=3)
            wkbp = pool(name="wkb", bufs=2)
            phA = pool(name="phA", bufs=2)
            aoutp = pool(name="aout", bufs=2)
            vpool = pool(name="vpool", bufs=2)
            opool = pool(name="opool", bufs=2)
            soft = pool(name="soft", bufs=2)
            dram = pool(name="dram", bufs=1, space="DRAM")
            ps_big = pool(name="ps_big", bufs=3, space="PSUM")
            ps_s = pool(name="ps_s", bufs=2, space="PSUM")
            ps_t = pool(name="ps_t", bufs=2, space="PSUM")
            ps_warm = pool(name="ps_warm", bufs=1, space="PSUM")

            # --- constants (sync ring) ---
            mask_sb = const.tile([P, P], F32)
            nc.sync.dma_start(mask_sb, maskt[:])
            ident_sb = const.tile([P, P], BF16)
            nc.sync.dma_start(ident_sb, ident[:])
            ones_sb = const.tile([1, P], BF16)
            nc.sync.dma_start(ones_sb, ones1[:])
            bv_sb = const.tile([P, D], BF16)
            nc.sync.dma_start(bv_sb, bvb[:])
            beta_f = const.tile([P, KO], F32)
            nc.sync.dma_start(beta_f, beta_col[:])
            beta_bf = const.tile([P, KO], BF16)
            nc.vector.tensor_copy(beta_bf, beta_f)

            # warm-up matmuls cover the first DMA-bound microseconds and
            # keep the clock governor fed before real work arrives
            for _ in range(120):
                wps = ps_warm.tile([P, P], F32, tag="warm", name="wps")
                nc.tensor.matmul(wps, lhsT=ident_sb, rhs=ident_sb, start=True, stop=True)

            # --- persistent SBUF intermediates ---
            xT = [
                xT_pool.tile([P, KO, nrows], BF16, name=f"xT{bi}")
                for bi, (_, nrows) in enumerate(BLOCKS)
            ]
            wqT_sl = wqp.tile([P, KO, SL], BF16, name="wqT_sl")
            S_all = sacc.tile([P, N_SUB, P], F32, name="S_all")
            wT_all = sacc.tile([P, N_SUB, P], BF16, name="wT_all")
            b_all = sacc.tile([P, N_SUB], BF16, name="b_all")
            bT_flat = sacc.tile([1, N_SUB * P], BF16, name="bT_flat")
            v0_all = v0pool.tile([P, N_SUB, V_W0], BF16, name="v0_all")

            # DRAM bounce buffers for the A AllGathers
            ag_in = [dram.tile([SL, 512], BF16, name=f"ag_in{g}") for g in range(MC)]
            ag_out = [
                dram.tile([D, 512], BF16, addr_space="Shared", name=f"ag_out{g}")
                for g in range(MC)
            ]
            ag_out_ap = [
                ag_out[g][:].rearrange("(kt p) m -> p kt m", p=P) for g in range(MC)
            ]

            # --- phase A: wq slice, wv0, x blocks (both HWDGE rings) ---
            for j in range(2):
                for s in range(4):
                    xt = phA.tile([P, 512], F32, tag="xt", name="wq_f", bufs=6)
                    eng = nc.sync if s % 2 == 0 else nc.scalar
                    eng.dma_start(
                        xt, wq_sl[j * P : (j + 1) * P, s * 512 : (s + 1) * 512]
                    )
                    xb = phA.tile([P, 512], BF16, tag="xb", name="wq_b")
                    nc.vector.tensor_copy(xb, xt)
                    for t in range(4):
                        ot = s * 4 + t
                        pst = ps_t.tile([P, P], BF16, tag="tr", name="pst")
                        nc.tensor.transpose(pst, xb[:, t * P : (t + 1) * P], ident_sb)
                        nc.vector.tensor_copy(wqT_sl[:, ot, j * P : (j + 1) * P], pst)

            # wv0: Wv cols 0:128 + beta column -> width 129
            wv0 = wv0p.tile([P, KO, V_W0 + 1], BF16, name="wv0")
            for k0 in range(KO):
                tmp = wtmp.tile([P, 1, 512], F32, tag="wtmp", name="wv0_tmp")[
                    :, :, :V_W0
                ]
                nc.sync.dma_start(tmp, wv_ap[:, k0 : k0 + 1, 0:V_W0])
                nc.vector.tensor_copy(wv0[:, k0 : k0 + 1, 0:V_W0], tmp)
            nc.vector.tensor_copy(wv0[:, :, V_W0], beta_bf)

            def phase_a_block(bi):
                row0, nrows = BLOCKS[bi]
                for s in range(4):
                    for rt in range(nrows // P):
                        r0 = row0 + rt * P
                        xt = phA.tile([P, 512], F32, tag="xt", name="xt", bufs=6)
                        eng = nc.sync if (s + rt) % 2 == 0 else nc.scalar
                        eng.dma_start(xt, xs[r0 : r0 + P, s * 512 : (s + 1) * 512])
                        xb = phA.tile([P, 512], BF16, tag="xb", name="xb")
                        nc.vector.tensor_copy(xb, xt)
                        for t in range(4):
                            kt = s * 4 + t
                            pst = ps_t.tile([P, P], BF16, tag="tr", name="pst")
                            nc.tensor.transpose(
                                pst, xb[:, t * P : (t + 1) * P], ident_sb
                            )
                            nc.vector.tensor_copy(
                                xT[bi][:, kt, rt * P : (rt + 1) * P], pst
                            )

            def v0_block(bi):
                for rs in range(4):
                    i = bi * 4 + rs
                    psv = ps_big.tile([P, 512], F32, tag="ps_big", name="psv")[
                        :, : V_W0 + 1
                    ]
                    for kt in range(KO):
                        nc.tensor.matmul(
                            psv,
                            lhsT=xT[bi][:, kt, rs * P : (rs + 1) * P],
                            rhs=wv0[:, kt, :],
                            start=(kt == 0),
                            stop=(kt == KO - 1),
                        )
                    nc.vector.tensor_copy(b_all[:, i : i + 1], psv[:, V_W0 : V_W0 + 1])
                    nc.vector.tensor_add(
                        v0_all[:, i, :], psv[:, 0:V_W0], bv_sb[:, 0:V_W0]
                    )

            for bi in range(4):
                phase_a_block(bi)

            # --- helpers for the g-loop ---
            def wk_quarter(g, wkT_g):
                # Wk rows [512g, 512g+512) -> WkT quarter [o, ot, d2-local]
                for sl in range(4):
                    d2_0 = 512 * g + 128 * sl
                    for qq in range(4):
                        wkf = wtmp.tile([P, 1, 512], F32, tag="wtmp", name="wkf")
                        wkf2 = wkf[:].rearrange("p a b -> p (a b)")
                        nc.sync.dma_start(
                            wkf2, Wk[d2_0 : d2_0 + P, qq * 512 : (qq + 1) * 512]
                        )
                        wkb = wkbp.tile([P, 512], BF16, tag="wkb", name="wkb")
                        nc.vector.tensor_copy(wkb, wkf2)
                        for t in range(4):
                            ot = qq * 4 + t
                            pst = ps_t.tile([P, P], BF16, tag="tr", name="pst")
                            nc.tensor.transpose(
                                pst, wkb[:, t * P : (t + 1) * P], ident_sb
                            )
                            nc.vector.tensor_copy(
                                wkT_g[:, ot, 128 * sl : 128 * sl + P], pst
                            )

            def a_slice_chunk(g, wkT_g):
                for db in range(2):
                    psA = ps_big.tile([P, 512], F32, tag="ps_big", name="psA")
                    for ot in range(KO):
                        nc.tensor.matmul(
                            psA,
                            lhsT=wqT_sl[:, ot, db * P : (db + 1) * P],
                            rhs=wkT_g[:, ot, :],
                            start=(ot == 0),
                            stop=(ot == KO - 1),
                        )
                    for hh in range(2):
                        aob = aoutp.tile([P, 256], BF16, tag="aout", name="aob")
                        nc.scalar.activation(
                            aob,
                            psA[:, hh * 256 : (hh + 1) * 256],
                            mybir.ActivationFunctionType.Identity,
                        )
                        nc.scalar.dma_start(
                            ag_in[g][db * P : (db + 1) * P, hh * 256 : (hh + 1) * 256],
                            aob,
                        )
                nc.gpsimd.collective_compute(
                    "AllGather",
                    mybir.AluOpType.bypass,
                    replica_groups=[list(range(N_CORES))],
                    ins=[ag_in[g].opt()],
                    outs=[ag_out[g].opt()],
                )

            def load_a_chunk(ach):
                g, half = ach // 2, ach % 2
                dst = apool.tile([P, KO, 256], BF16, tag="ach", name="a_sb")
                nc.scalar.dma_start(
                    dst, ag_out_ap[g][:, :, half * 256 : (half + 1) * 256]
                )
                return dst

            uT = {}  # ach -> [P, 2, R] tile (uTp pool, bufs=3)

            def emit_s_pair(ach, i0, single=False):
                last = ach == ACH - 1
                for i in ((i0,) if single else (i0, i0 + 1)):
                    bi, rs = i // 4, i % 4
                    pss = ps_s.tile([P, P], F32, tag="pss", name="pss")
                    for ml in range(2):
                        mt = ach * 2 + ml
                        nc.tensor.matmul(
                            pss,
                            lhsT=uT[ach][:, ml, i * P : (i + 1) * P],
                            rhs=xT[bi][:, mt, rs * P : (rs + 1) * P],
                            start=(ml == 0),
                            stop=(ml == 1 and not last),
                        )
                    if last:
                        nc.tensor.matmul(
                            pss,
                            lhsT=ones_sb,
                            rhs=bT_flat[:, i * P : (i + 1) * P],
                            start=False,
                            stop=True,
                        )
                    if ach == 0:
                        nc.vector.tensor_copy(S_all[:, i, :], pss)
                    else:
                        nc.vector.tensor_add(S_all[:, i, :], S_all[:, i, :], pss)

            pending_s = None

            def u_chunk(ach, a_sb):
                nonlocal pending_s
                uT[ach] = uTp.tile([P, 2, R], BF16, tag="uT", name=f"uT{ach}")
                for j in range(8):
                    ml, bi = j // 4, j % 4
                    row0, nrows = BLOCKS[bi]
                    psu = ps_big.tile([P, 512], F32, tag="ps_big", name="psu")
                    for kt in range(KO):
                        nc.tensor.matmul(
                            psu,
                            lhsT=a_sb[:, kt, ml * P : (ml + 1) * P],
                            rhs=xT[bi][:, kt, :],
                            start=(kt == 0),
                            stop=(kt == KO - 1),
                        )
                    nc.scalar.activation(
                        uT[ach][:, ml, row0 : row0 + nrows],
                        psu,
                        mybir.ActivationFunctionType.Identity,
                    )
                    if pending_s is not None:
                        emit_s_pair(pending_s, 2 * j)
                pending_s = ach
                if pending_s - 2 in uT:
                    del uT[pending_s - 2]

            # --- pipelined A production: AllGather runs 2+ sections ahead of
            # the u-chunks that consume it (AG end-to-end is ~50us) ---
            a_tiles = {}

            def g_section(g):
                wkT_g = wkv.tile([P, KO, 512], BF16, tag="wkv", name=f"wkT{g}")
                wk_quarter(g, wkT_g)
                a_slice_chunk(g, wkT_g)

            g_section(0)
            g_section(1)
            # V c0 + b extraction fills the first AllGather window
            for bi in range(4):
                v0_block(bi)
            for i in range(N_SUB):
                pst = ps_t.tile([P, P], BF16, tag="tr", name="pst_b")
                nc.tensor.transpose(pst[:1, :], b_all[:, i : i + 1], ident_sb)
                nc.vector.tensor_copy(bT_flat[:, i * P : (i + 1) * P], pst[:1, :])

            a_tiles[0] = load_a_chunk(0)
            a_tiles[1] = load_a_chunk(1)
            g_section(2)
            u_chunk(0, a_tiles.pop(0))
            u_chunk(1, a_tiles.pop(1))
            a_tiles[2] = load_a_chunk(2)
            a_tiles[3] = load_a_chunk(3)
            g_section(3)
            u_chunk(2, a_tiles.pop(2))
            a_tiles[4] = load_a_chunk(4)
            u_chunk(3, a_tiles.pop(3))
            a_tiles[5] = load_a_chunk(5)
            u_chunk(4, a_tiles.pop(4))
            a_tiles[6] = load_a_chunk(6)
            u_chunk(5, a_tiles.pop(5))
            a_tiles[7] = load_a_chunk(7)
            u_chunk(6, a_tiles.pop(6))
            u_chunk(7, a_tiles.pop(7))

            # --- tail: last S partials + softmax fused into the V c1 pass ---
            def emit_softmax(i):
                tmask = soft.tile([P, P], F32, tag="tmask")
                nc.vector.tensor_add(tmask, S_all[:, i, :], mask_sb)
                e = soft.tile([P, P], F32, tag="e")
                ssum = soft.tile([P, 1], F32, tag="ssum")
                nc.scalar.activation(
                    e, tmask, mybir.ActivationFunctionType.Exp,
                    scale=float(SCALE), accum_out=ssum,
                )
                rcp = soft.tile([P, 1], F32, tag="rcp")
                nc.vector.reciprocal(rcp, ssum)
                wsb = soft.tile([P, P], BF16, tag="wsb")
                nc.vector.tensor_scalar_mul(wsb, e, rcp)
                pstw = ps_t.tile([P, P], BF16, tag="tr", name="pstw")
                nc.tensor.transpose(pstw, wsb, ident_sb)
                nc.vector.tensor_copy(wT_all[:, i, :], pstw)

            def load_wv_chunk(c):
                col0, width = V_CHUNKS[c]
                dst = wkv.tile([P, KO, 512], BF16, tag="wkv", name="wv_sb")
                for k0 in range(KO):
                    tmp = wtmp.tile([P, 1, 512], F32, tag="wtmp", name="wv_tmp")[
                        :, :, :width
                    ]
                    nc.sync.dma_start(tmp, wv_ap[:, k0 : k0 + 1, col0 : col0 + width])
                    nc.vector.tensor_copy(dst[:, k0 : k0 + 1, 0:width], tmp)
                return dst

            def emit_o(v_sb, i, col0, width):
                pso = ps_big.tile([P, 512], F32, tag="ps_big", name="pso")[:, :width]
                nc.tensor.matmul(
                    pso, lhsT=wT_all[:, i, :], rhs=v_sb, start=True, stop=True
                )
                o_sb = opool.tile([P, 512], F32, tag="o", name="o_sb")[:, :width]
                nc.vector.tensor_copy(o_sb, pso)
                nc.scalar.dma_start(out[i * P : (i + 1) * P, col0 : col0 + width], o_sb)

            # V c1 chains keep the PE dense while the last S partials and
            # the softmaxes drain on DVE/ACT; O c0+c1 follow one step behind
            wv_tiles = {0: load_wv_chunk(0)}
            col1, wid1 = V_CHUNKS[0]
            v1_tiles = {}
            prev = None
            for i in range(N_SUB):
                emit_s_pair(ACH - 1, i, single=True)
                psv = ps_big.tile([P, 512], F32, tag="ps_big", name="psv1")[:, :wid1]
                bi, rs = i // 4, i % 4
                for kt in range(KO):
                    nc.tensor.matmul(
                        psv,
                        lhsT=xT[bi][:, kt, rs * P : (rs + 1) * P],
                        rhs=wv_tiles[0][:, kt, 0:wid1],
                        start=(kt == 0),
                        stop=(kt == KO - 1),
                    )
                v_sb = vpool.tile([P, 512], BF16, tag="v", name="v_sb")[:, :wid1]
                nc.vector.tensor_add(v_sb, psv, bv_sb[:, col1 : col1 + wid1])
                v1_tiles[i] = v_sb
                if prev is not None:
                    emit_softmax(prev)
                    pso = ps_big.tile([P, 512], F32, tag="ps_big", name="pso0")[
                        :, :V_W0
                    ]
                    nc.tensor.matmul(
                        pso, lhsT=wT_all[:, prev, :], rhs=v0_all[:, prev, :],
                        start=True, stop=True,
                    )
                    o_sb = opool.tile([P, 512], F32, tag="o", name="o_sb0")[:, :V_W0]
                    nc.vector.tensor_copy(o_sb, pso)
                    nc.scalar.dma_start(out[prev * P : (prev + 1) * P, 0:V_W0], o_sb)
                    emit_o(v1_tiles.pop(prev), prev, col1, wid1)
                prev = i
            pending_s = None
            emit_softmax(prev)
            pso = ps_big.tile([P, 512], F32, tag="ps_big", name="pso0")[:, :V_W0]
            nc.tensor.matmul(
                pso, lhsT=wT_all[:, prev, :], rhs=v0_all[:, prev, :],
                start=True, stop=True,
            )
            o_sb = opool.tile([P, 512], F32, tag="o", name="o_sb0")[:, :V_W0]
            nc.vector.tensor_copy(o_sb, pso)
            nc.scalar.dma_start(out[prev * P : (prev + 1) * P, 0:V_W0], o_sb)
            emit_o(v1_tiles.pop(prev), prev, col1, wid1)
            wv_tiles.pop(0)

            # --- pass 2 remainder: V chunks c2..c4 + O ---
            pending_o = None
            wv_tiles[1] = load_wv_chunk(1)
            for c in range(1, len(V_CHUNKS)):
                col0, width = V_CHUNKS[c]
                if c + 1 < len(V_CHUNKS) and (c + 1) not in wv_tiles:
                    wv_tiles[c + 1] = load_wv_chunk(c + 1)
                wv_sb = wv_tiles.pop(c)
                for bi, (row0, nrows) in enumerate(BLOCKS):
                    for rs in range(4):
                        i = bi * 4 + rs
                        psv = ps_big.tile([P, 512], F32, tag="ps_big", name="psv2")[
                            :, :width
                        ]
                        for kt in range(KO):
                            nc.tensor.matmul(
                                psv,
                                lhsT=xT[bi][:, kt, rs * P : (rs + 1) * P],
                                rhs=wv_sb[:, kt, 0:width],
                                start=(kt == 0),
                                stop=(kt == KO - 1),
                            )
                        v_sb = vpool.tile([P, 512], BF16, tag="v", name="v_sb")[
                            :, :width
                        ]
                        nc.vector.tensor_add(v_sb, psv, bv_sb[:, col0 : col0 + width])
                        if pending_o is not None:
                            emit_o(*pending_o)
                        pending_o = (v_sb, i, col0, width)
            if pending_o is not None:
                emit_o(*pending_o)
                pending_o = None

    nc.compile()
    return nc


_CACHED = {}


def host_constants():
    mask = np.full((P, P), -1e9, dtype=np.float32)
    for g in range(P // H):
        mask[g * H : (g + 1) * H, g * H : (g + 1) * H] = 0.0
    identity = np.eye(P, dtype=ml_dtypes.bfloat16)
    ones_row = np.ones((1, P), dtype=ml_dtypes.bfloat16)
    return mask, identity, ones_row


def make_in_maps(x, Wq, bq, Wk, bk, Wv, bv):
    x = np.ascontiguousarray(np.asarray(x, dtype=np.float32))
    Wq = np.ascontiguousarray(np.asarray(Wq, dtype=np.float32))
    Wk = np.ascontiguousarray(np.asarray(Wk, dtype=np.float32))
    Wv = np.ascontiguousarray(np.asarray(Wv, dtype=np.float32))
    bq = np.asarray(bq, dtype=np.float32)
    bv = np.asarray(bv, dtype=np.float32)

    mask, identity, ones_row = host_constants()
    beta = Wk @ bq  # [D]; surviving score-bias term is b = x @ beta
    beta_col = np.ascontiguousarray(beta.reshape(KO, P).T)
    bvb = np.ascontiguousarray(
        np.broadcast_to(bv.astype(ml_dtypes.bfloat16), (P, D))
    )

    in_maps = []
    for i in range(N_CORES):
        in_maps.append(
            {
                "xs": x[i * R : (i + 1) * R],
                "Wk": Wk,
                "Wv": Wv,
                "wq_sl": np.ascontiguousarray(Wq[i * SL : (i + 1) * SL]),
                "beta_col": beta_col,
                "bvb": bvb,
                "maskt": mask,
                "ident": identity,
                "ones1": ones_row,
            }
        )
    return in_maps


def kernel(x, Wq, bq, Wk, bk, Wv, bv):
    if "nc" not in _CACHED:
        _CACHED["nc"] = build_program()
    nc = _CACHED["nc"]
    in_maps = make_in_maps(x, Wq, bq, Wk, bk, Wv, bv)
    res = run_bass_kernel_spmd(nc, in_maps, list(range(N_CORES)))
    return np.concatenate([res.results[i]["out"] for i in range(N_CORES)], axis=0)


# revision 47
# speedup vs baseline: 1.0925x; 1.0673x over previous
"""Trainium2 Bass kernel for nn_MultiHeadAttention_63986422775834.

Computation (see harness reference):
    q = x @ Wq + bq; k = x @ Wk + bk; v = x @ Wv + bv          # [N, D]
    group rows by 8: scores[b,h,g] = q[8b+h] . k[8b+g] / sqrt(D)
    w = softmax(scores, axis=-1);  out[8b+h] = sum_g w[b,h,g] * v[8b+g]

Algebraic restructuring: scores[i,j] = x_i (Wq Wk^T) x_j^T + const_i
+ x_j.(Wk bq) + const, and the const_i terms drop under softmax; so with
A = Wq Wk^T (shared by ALL rows on ALL cores) and beta = Wk bq:
    scores[i,j] = u_i . x_j + b_j,   u = x @ A,   b = x @ beta
This replaces the two q/k projection GEMMs (2 x 2048^3 MACs/core) with
ONE GEMM u = x @ A plus a SHARDED A: each core computes a 256-row slice
of A and 4 column-group AllGathers share it (~7us each, hidden).  bv
rides through softmax (rows sum to 1) folded into the V tiles; b rides
as an extra appended column on the V c0 chunk and enters each score
PSUM chain as a rank-1 matmul.

Schedule (the PE clock governor rewards an always-busy tensor engine,
so the kernel is ordered to keep the PE stream gap-free from ~10us):
  phase A: x at full DMA priority -> xT transposes + V c0 (129 wide,
           with the beta column) keep the PE fed while Wk streams.
  g-loop:  per 512-row quarter of Wk: transposes -> A-slice chunk ->
           AllGather_g; u-chunks of quarter g-1 run between, so no PE
           idle while a collective is in flight.  S partials of a-chunk
           n interleave into the u-chains of chunk n+1.
  tail:    last S partials + rank-1 b + softmax + O c0, then V c1..c4
           (512-wide) with O, all dense.
Stores (A bounce, outputs) ride the scalar-engine DMA ring so they
cannot block the load stream on the sync ring.
"""

import sys

sys.path.insert(0, "/opt/trn_rl_repo")

import numpy as np
import ml_dtypes

import concourse.mybir as mybir
import concourse.tile as tile
from concourse import bacc
from concourse.bass_utils import run_bass_kernel_spmd

# problem shape (hardcoded per contract)
N_FULL = 16384
D = 2048
H = 8
N_CORES = 8
R = N_FULL // N_CORES  # rows per core = 2048
P = 128
KO = D // P  # 16 k-subtiles along d_in
SL = D // N_CORES  # A rows computed per core = 256
SCALE = 1.0 / np.sqrt(np.float32(D))

BF16 = mybir.dt.bfloat16
F32 = mybir.dt.float32

BLOCKS = [(0, 512), (512, 512), (1024, 512), (1536, 512)]
N_SUB = R // P  # 16 row subtiles
MC = 4  # Wk quarters == A column groups == AllGathers
ACH = 8  # gathered-A chunks of 256 cols (2 m-tiles each)
V_W0 = 128  # V c0 real width (col 128 carries b = x.beta)
V_CHUNKS = [(128, 512), (640, 512), (1152, 512), (1664, 384)]


def build_program():
    nc = bacc.Bacc("TRN2", target_bir_lowering=False, debug=False, num_devices=N_CORES)

    xs = nc.dram_tensor("xs", [R, D], F32, kind="ExternalInput")
    Wk = nc.dram_tensor("Wk", [D, D], F32, kind="ExternalInput")
    Wv = nc.dram_tensor("Wv", [D, D], F32, kind="ExternalInput")
    wq_sl = nc.dram_tensor("wq_sl", [SL, D], F32, kind="ExternalInput")
    beta_col = nc.dram_tensor("beta_col", [P, KO], F32, kind="ExternalInput")
    bvb = nc.dram_tensor("bvb", [P, D], BF16, kind="ExternalInput")
    maskt = nc.dram_tensor("maskt", [P, P], F32, kind="ExternalInput")
    ident = nc.dram_tensor("ident", [P, P], BF16, kind="ExternalInput")
    ones1 = nc.dram_tensor("ones1", [1, P], BF16, kind="ExternalInput")
    out = nc.dram_tensor("out", [R, D], F32, kind="ExternalOutput")

    wv_ap = Wv[:].rearrange("(ko p) n -> p ko n", p=P)

    from contextlib import ExitStack

    with tile.TileContext(nc) as tc:
        with ExitStack() as stack:
            pool = lambda *a, **kw: stack.enter_context(tc.tile_pool(*a, **kw))
            const = pool(name="const", bufs=1)
            xT_pool = pool(name="xT", bufs=1)
            wkv = pool(name="wkv", bufs=2)  # WkT quarters -> wv chunks
            wv0p = pool(name="wv0p", bufs=1)
            uTp = pool(name="uTp", bufs=3)  # per-a-chunk uT tiles
            apool = pool(name="achunk", bufs=2)
            wqp = pool(name="wqp", bufs=1)
            v0pool = pool(name="v0", bufs=1)
            sacc = pool(name="sacc", bufs=1)
            wtmp = pool(name="wtmp", bufs=3)
            wkbp = pool(name="wkb", bufs=2)
            phA = pool(name="phA", bufs=2)
            aoutp = pool(name="aout", bufs=2)
            vpool = pool(name="vpool", bufs=2)
            opool = pool(name="opool", bufs=2)
            soft = pool(name="soft", bufs=2)
            dram = pool(name="dram", bufs=1, space="DRAM")
            ps_big = pool(name="ps_big", bufs=3, space="PSUM")
            ps_s = pool(name="ps_s", bufs=2, space="PSUM")
            ps_t = pool(name="ps_t", bufs=2, space="PSUM")
            ps_warm = pool(name="ps_warm", bufs=1, space="PSUM")

            # --- constants (sync ring) ---
            mask_sb = const.tile([P, P], F32)
            nc.sync.dma_start(mask_sb, maskt[:])
            ident_sb = const.tile([P, P], BF16)
            nc.sync.dma_start(ident_sb, ident[:])
            ones_sb = const.tile([1, P], BF16)
            nc.sync.dma_start(ones_sb, ones1[:])
            bv_sb = const.tile([P, D], BF16)
            nc.sync.dma_start(bv_sb, bvb[:])
            beta_f = const.tile([P, KO], F32)
            nc.sync.dma_start(beta_f, beta_col[:])
            beta_bf = const.tile([P, KO], BF16)
            nc.vector.tensor_copy(beta_bf, beta_f)

            # warm-up matmuls cover the first DMA-bound microseconds and
            # keep the clock governor fed before real work arrives
            for _ in range(120):
                wps = ps_warm.tile([P, P], F32, tag="warm", name="wps")
                nc.tensor.matmul(wps, lhsT=ident_sb, rhs=ident_sb, start=True, stop=True)

            # --- persistent SBUF intermediates ---
            xT = [
                xT_pool.tile([P, KO, nrows], BF16, name=f"xT{bi}")
                for bi, (_, nrows) in enumerate(BLOCKS)
            ]
            wqT_sl = wqp.tile([P, KO, SL], BF16, name="wqT_sl")
            S_all = sacc.tile([P, N_SUB, P], F32, name="S_all")
            wT_all = sacc.tile([P, N_SUB, P], BF16, name="wT_all")
            b_all = sacc.tile([P, N_SUB], BF16, name="b_all")
            bT_flat = sacc.tile([1, N_SUB * P], BF16, name="bT_flat")
            v0_all = v0pool.tile([P, N_SUB, V_W0], BF16, name="v0_all")

            # DRAM bounce buffers for the A AllGathers
            ag_in = [dram.tile([SL, 512], BF16, name=f"ag_in{g}") for g in range(MC)]
            ag_out = [
                dram.tile([D, 512], BF16, addr_space="Shared", name=f"ag_out{g}")
                for g in range(MC)
            ]
            ag_out_ap = [
                ag_out[g][:].rearrange("(kt p) m -> p kt m", p=P) for g in range(MC)
            ]

            # tiny warm-up AllGather: pays the collective cold-start cost
            # (~60us on the first gather) at t~0, off the critical path
            wag_in = dram.tile([16, 16], F32, name="wag_in")
            wag_out = dram.tile([N_CORES * 16, 16], F32, addr_space="Shared", name="wag_out")
            nc.gpsimd.collective_compute(
                "AllGather",
                mybir.AluOpType.bypass,
                replica_groups=[list(range(N_CORES))],
                ins=[wag_in.opt()],
                outs=[wag_out.opt()],
            )

            # --- phase A: wq slice, wv0, x blocks (both HWDGE rings) ---
            for j in range(2):
                for s in range(4):
                    xt = phA.tile([P, 512], F32, tag="xt", name="wq_f", bufs=6)
                    eng = nc.sync if s % 2 == 0 else nc.scalar
                    eng.dma_start(
                        xt, wq_sl[j * P : (j + 1) * P, s * 512 : (s + 1) * 512]
                    )
                    xb = phA.tile([P, 512], BF16, tag="xb", name="wq_b")
                    nc.vector.tensor_copy(xb, xt)
                    for t in range(4):
                        ot = s * 4 + t
                        pst = ps_t.tile([P, P], BF16, tag="tr", name="pst")
                        nc.tensor.transpose(pst, xb[:, t * P : (t + 1) * P], ident_sb)
                        nc.vector.tensor_copy(wqT_sl[:, ot, j * P : (j + 1) * P], pst)

            # wv0: Wv cols 0:128 + beta column -> width 129
            wv0 = wv0p.tile([P, KO, V_W0 + 1], BF16, name="wv0")
            for k0 in range(KO):
                tmp = wtmp.tile([P, 1, 512], F32, tag="wtmp", name="wv0_tmp")[
                    :, :, :V_W0
                ]
                nc.sync.dma_start(tmp, wv_ap[:, k0 : k0 + 1, 0:V_W0])
                nc.vector.tensor_copy(wv0[:, k0 : k0 + 1, 0:V_W0], tmp)
            nc.vector.tensor_copy(wv0[:, :, V_W0], beta_bf)

            def phase_a_block(bi):
                row0, nrows = BLOCKS[bi]
                for s in range(4):
                    for rt in range(nrows // P):
                        r0 = row0 + rt * P
                        xt = phA.tile([P, 512], F32, tag="xt", name="xt", bufs=6)
                        eng = nc.sync if (s + rt) % 2 == 0 else nc.scalar
                        eng.dma_start(xt, xs[r0 : r0 + P, s * 512 : (s + 1) * 512])
                        xb = phA.tile([P, 512], BF16, tag="xb", name="xb")
                        nc.vector.tensor_copy(xb, xt)
                        for t in range(4):
                            kt = s * 4 + t
                            pst = ps_t.tile([P, P], BF16, tag="tr", name="pst")
                            nc.tensor.transpose(
                                pst, xb[:, t * P : (t + 1) * P], ident_sb
                            )
                            nc.vector.tensor_copy(
                                xT[bi][:, kt, rt * P : (rt + 1) * P], pst
                            )

            def v0_block(bi):
                for rs in range(4):
                    i = bi * 4 + rs
                    psv = ps_big.tile([P, 512], F32, tag="ps_big", name="psv")[
                        :, : V_W0 + 1
                    ]
                    for kt in range(KO):
                        nc.tensor.matmul(
                            psv,
                            lhsT=xT[bi][:, kt, rs * P : (rs + 1) * P],
                            rhs=wv0[:, kt, :],
                            start=(kt == 0),
                            stop=(kt == KO - 1),
                        )
                    nc.vector.tensor_copy(b_all[:, i : i + 1], psv[:, V_W0 : V_W0 + 1])
                    nc.vector.tensor_add(
                        v0_all[:, i, :], psv[:, 0:V_W0], bv_sb[:, 0:V_W0]
                    )

            # (phase-A x blocks are emitted inside the pipeline below, after
            # the first A-slice section, so AG0 completes under phase A)

            # --- helpers for the g-loop ---
            def wk_quarter(g, wkT_g):
                # Wk rows [512g, 512g+512) -> WkT quarter [o, ot, d2-local]
                for sl in range(4):
                    d2_0 = 512 * g + 128 * sl
                    for qq in range(4):
                        wkf = wtmp.tile([P, 1, 512], F32, tag="wtmp", name="wkf")
                        wkf2 = wkf[:].rearrange("p a b -> p (a b)")
                        nc.sync.dma_start(
                            wkf2, Wk[d2_0 : d2_0 + P, qq * 512 : (qq + 1) * 512]
                        )
                        wkb = wkbp.tile([P, 512], BF16, tag="wkb", name="wkb")
                        nc.vector.tensor_copy(wkb, wkf2)
                        for t in range(4):
                            ot = qq * 4 + t
                            pst = ps_t.tile([P, P], BF16, tag="tr", name="pst")
                            nc.tensor.transpose(
                                pst, wkb[:, t * P : (t + 1) * P], ident_sb
                            )
                            nc.vector.tensor_copy(
                                wkT_g[:, ot, 128 * sl : 128 * sl + P], pst
                            )

            def a_slice_chunk(g, wkT_g):
                for db in range(2):
                    psA = ps_big.tile([P, 512], F32, tag="ps_big", name="psA")
                    for ot in range(KO):
                        nc.tensor.matmul(
                            psA,
                            lhsT=wqT_sl[:, ot, db * P : (db + 1) * P],
                            rhs=wkT_g[:, ot, :],
                            start=(ot == 0),
                            stop=(ot == KO - 1),
                        )
                    for hh in range(2):
                        aob = aoutp.tile([P, 256], BF16, tag="aout", name="aob")
                        nc.scalar.activation(
                            aob,
                            psA[:, hh * 256 : (hh + 1) * 256],
                            mybir.ActivationFunctionType.Identity,
                        )
                        nc.scalar.dma_start(
                            ag_in[g][db * P : (db + 1) * P, hh * 256 : (hh + 1) * 256],
                            aob,
                        )
                nc.gpsimd.collective_compute(
                    "AllGather",
                    mybir.AluOpType.bypass,
                    replica_groups=[list(range(N_CORES))],
                    ins=[ag_in[g].opt()],
                    outs=[ag_out[g].opt()],
                )

            def load_a_chunk(ach):
                g, half = ach // 2, ach % 2
                dst = apool.tile([P, KO, 256], BF16, tag="ach", name="a_sb")
                nc.scalar.dma_start(
                    dst, ag_out_ap[g][:, :, half * 256 : (half + 1) * 256]
                )
                return dst

            uT = {}  # ach -> [P, 2, R] tile (uTp pool, bufs=3)

            def emit_s_pair(ach, i0, single=False):
                last = ach == ACH - 1
                for i in ((i0,) if single else (i0, i0 + 1)):
                    bi, rs = i // 4, i % 4
                    pss = ps_s.tile([P, P], F32, tag="pss", name="pss")
                    for ml in range(2):
                        mt = ach * 2 + ml
                        nc.tensor.matmul(
                            pss,
                            lhsT=uT[ach][:, ml, i * P : (i + 1) * P],
                            rhs=xT[bi][:, mt, rs * P : (rs + 1) * P],
                            start=(ml == 0),
                            stop=(ml == 1 and not last),
                        )
                    if last:
                        nc.tensor.matmul(
                            pss,
                            lhsT=ones_sb,
                            rhs=bT_flat[:, i * P : (i + 1) * P],
                            start=False,
                            stop=True,
                        )
                    if ach == 0:
                        nc.vector.tensor_copy(S_all[:, i, :], pss)
                    else:
                        nc.vector.tensor_add(S_all[:, i, :], S_all[:, i, :], pss)

            pending_s = None

            def u_chunk(ach, a_sb):
                nonlocal pending_s
                uT[ach] = uTp.tile([P, 2, R], BF16, tag="uT", name=f"uT{ach}")
                for j in range(8):
                    ml, bi = j // 4, j % 4
                    row0, nrows = BLOCKS[bi]
                    psu = ps_big.tile([P, 512], F32, tag="ps_big", name="psu")
                    for kt in range(KO):
                        nc.tensor.matmul(
                            psu,
                            lhsT=a_sb[:, kt, ml * P : (ml + 1) * P],
                            rhs=xT[bi][:, kt, :],
                            start=(kt == 0),
                            stop=(kt == KO - 1),
                        )
                    nc.scalar.activation(
                        uT[ach][:, ml, row0 : row0 + nrows],
                        psu,
                        mybir.ActivationFunctionType.Identity,
                    )
                    if pending_s is not None:
                        emit_s_pair(pending_s, 2 * j)
                pending_s = ach
                if pending_s - 2 in uT:
                    del uT[pending_s - 2]

            # --- pipelined A production: AllGather runs 2+ sections ahead of
            # the u-chunks that consume it (AG end-to-end is ~50us) ---
            a_tiles = {}

            def g_section(g):
                wkT_g = wkv.tile([P, KO, 512], BF16, tag="wkv", name=f"wkT{g}")
                wk_quarter(g, wkT_g)
                a_slice_chunk(g, wkT_g)

            g_section(0)  # Wk q0 is right after wq_sl in the DMA ring
            for bi in range(4):
                phase_a_block(bi)
            g_section(1)
            # V c0 + b extraction fills the remaining AllGather window
            for bi in range(4):
                v0_block(bi)
            for i in range(N_SUB):
                pst = ps_t.tile([P, P], BF16, tag="tr", name="pst_b")
                nc.tensor.transpose(pst[:1, :], b_all[:, i : i + 1], ident_sb)
                nc.vector.tensor_copy(bT_flat[:, i * P : (i + 1) * P], pst[:1, :])

            a_tiles[0] = load_a_chunk(0)
            a_tiles[1] = load_a_chunk(1)
            g_section(2)
            u_chunk(0, a_tiles.pop(0))
            u_chunk(1, a_tiles.pop(1))
            a_tiles[2] = load_a_chunk(2)
            a_tiles[3] = load_a_chunk(3)
            g_section(3)
            u_chunk(2, a_tiles.pop(2))
            a_tiles[4] = load_a_chunk(4)
            u_chunk(3, a_tiles.pop(3))
            a_tiles[5] = load_a_chunk(5)
            u_chunk(4, a_tiles.pop(4))
            a_tiles[6] = load_a_chunk(6)
            u_chunk(5, a_tiles.pop(5))
            a_tiles[7] = load_a_chunk(7)
            u_chunk(6, a_tiles.pop(6))
            u_chunk(7, a_tiles.pop(7))

            # --- tail: last S partials + softmax fused into the V c1 pass ---
            def emit_softmax(i):
                tmask = soft.tile([P, P], F32, tag="tmask")
                nc.vector.tensor_add(tmask, S_all[:, i, :], mask_sb)
                e = soft.tile([P, P], F32, tag="e")
                ssum = soft.tile([P, 1], F32, tag="ssum")
                nc.scalar.activation(
                    e, tmask, mybir.ActivationFunctionType.Exp,
                    scale=float(SCALE), accum_out=ssum,
                )
                rcp = soft.tile([P, 1], F32, tag="rcp")
                nc.vector.reciprocal(rcp, ssum)
                wsb = soft.tile([P, P], BF16, tag="wsb")
                nc.vector.tensor_scalar_mul(wsb, e, rcp)
                pstw = ps_t.tile([P, P], BF16, tag="tr", name="pstw")
                nc.tensor.transpose(pstw, wsb, ident_sb)
                nc.vector.tensor_copy(wT_all[:, i, :], pstw)

            def load_wv_chunk(c):
                col0, width = V_CHUNKS[c]
                dst = wkv.tile([P, KO, 512], BF16, tag="wkv", name="wv_sb")
                for k0 in range(KO):
                    tmp = wtmp.tile([P, 1, 512], F32, tag="wtmp", name="wv_tmp")[
                        :, :, :width
                    ]
                    nc.sync.dma_start(tmp, wv_ap[:, k0 : k0 + 1, col0 : col0 + width])
                    nc.vector.tensor_copy(dst[:, k0 : k0 + 1, 0:width], tmp)
                return dst

            def emit_o(v_sb, i, col0, width):
                pso = ps_big.tile([P, 512], F32, tag="ps_big", name="pso")[:, :width]
                nc.tensor.matmul(
                    pso, lhsT=wT_all[:, i, :], rhs=v_sb, start=True, stop=True
                )
                o_sb = opool.tile([P, 512], F32, tag="o", name="o_sb")[:, :width]
                nc.vector.tensor_copy(o_sb, pso)
                nc.scalar.dma_start(out[i * P : (i + 1) * P, col0 : col0 + width], o_sb)

            # V c1 chains keep the PE dense while the last S partials and
            # the softmaxes drain on DVE/ACT; O c0+c1 follow one step behind
            wv_tiles = {0: load_wv_chunk(0)}
            col1, wid1 = V_CHUNKS[0]
            v1_tiles = {}
            prev = None
            for i in range(N_SUB):
                emit_s_pair(ACH - 1, i, single=True)
                psv = ps_big.tile([P, 512], F32, tag="ps_big", name="psv1")[:, :wid1]
                bi, rs = i // 4, i % 4
                for kt in range(KO):
                    nc.tensor.matmul(
                        psv,
                        lhsT=xT[bi][:, kt, rs * P : (rs + 1) * P],
                        rhs=wv_tiles[0][:, kt, 0:wid1],
                        start=(kt == 0),
                        stop=(kt == KO - 1),
                    )
                v_sb = vpool.tile([P, 512], BF16, tag="v", name="v_sb")[:, :wid1]
                nc.vector.tensor_add(v_sb, psv, bv_sb[:, col1 : col1 + wid1])
                v1_tiles[i] = v_sb
                if prev is not None:
                    emit_softmax(prev)
                    pso = ps_big.tile([P, 512], F32, tag="ps_big", name="pso0")[
                        :, :V_W0
                    ]
                    nc.tensor.matmul(
                        pso, lhsT=wT_all[:, prev, :], rhs=v0_all[:, prev, :],
                        start=True, stop=True,
                    )
                    o_sb = opool.tile([P, 512], F32, tag="o", name="o_sb0")[:, :V_W0]
                    nc.vector.tensor_copy(o_sb, pso)
                    nc.scalar.dma_start(out[prev * P : (prev + 1) * P, 0:V_W0], o_sb)
                    emit_o(v1_tiles.pop(prev), prev, col1, wid1)
                prev = i
            pending_s = None
            emit_softmax(prev)
            pso = ps_big.tile([P, 512], F32, tag="ps_big", name="pso0")[:, :V_W0]
            nc.tensor.matmul(
                pso, lhsT=wT_all[:, prev, :], rhs=v0_all[:, prev, :],
                start=True, stop=True,
            )
            o_sb = opool.tile([P, 512], F32, tag="o", name="o_sb0")[:, :V_W0]
            nc.vector.tensor_copy(o_sb, pso)
            nc.scalar.dma_start(out[prev * P : (prev + 1) * P, 0:V_W0], o_sb)
            emit_o(v1_tiles.pop(prev), prev, col1, wid1)
            wv_tiles.pop(0)

            # --- pass 2 remainder: V chunks c2..c4 + O ---
            pending_o = None
            wv_tiles[1] = load_wv_chunk(1)
            for c in range(1, len(V_CHUNKS)):
                col0, width = V_CHUNKS[c]
                if c + 1 < len(V_CHUNKS) and (c + 1) not in wv_tiles:
                    wv_tiles[c + 1] = load_wv_chunk(c + 1)
                wv_sb = wv_tiles.pop(c)
                for bi, (row0, nrows) in enumerate(BLOCKS):
                    for rs in range(4):
                        i = bi * 4 + rs
                        psv = ps_big.tile([P, 512], F32, tag="ps_big", name="psv2")[
                            :, :width
                        ]
                        for kt in range(KO):
                            nc.tensor.matmul(
                                psv,
                                lhsT=xT[bi][:, kt, rs * P : (rs + 1) * P],
                                rhs=wv_sb[:, kt, 0:width],
                                start=(kt == 0),
                                stop=(kt == KO - 1),
                            )
                        v_sb = vpool.tile([P, 512], BF16, tag="v", name="v_sb")[
                            :, :width
                        ]
                        nc.vector.tensor_add(v_sb, psv, bv_sb[:, col0 : col0 + width])
                        if pending_o is not None:
                            emit_o(*pending_o)
                        pending_o = (v_sb, i, col0, width)
            if pending_o is not None:
                emit_o(*pending_o)
                pending_o = None

    nc.compile()
    return nc


_CACHED = {}


def host_constants():
    mask = np.full((P, P), -1e9, dtype=np.float32)
    for g in range(P // H):
        mask[g * H : (g + 1) * H, g * H : (g + 1) * H] = 0.0
    identity = np.eye(P, dtype=ml_dtypes.bfloat16)
    ones_row = np.ones((1, P), dtype=ml_dtypes.bfloat16)
    return mask, identity, ones_row


def make_in_maps(x, Wq, bq, Wk, bk, Wv, bv):
    x = np.ascontiguousarray(np.asarray(x, dtype=np.float32))
    Wq = np.ascontiguousarray(np.asarray(Wq, dtype=np.float32))
    Wk = np.ascontiguousarray(np.asarray(Wk, dtype=np.float32))
    Wv = np.ascontiguousarray(np.asarray(Wv, dtype=np.float32))
    bq = np.asarray(bq, dtype=np.float32)
    bv = np.asarray(bv, dtype=np.float32)

    mask, identity, ones_row = host_constants()
    beta = Wk @ bq  # [D]; surviving score-bias term is b = x @ beta
    beta_col = np.ascontiguousarray(beta.reshape(KO, P).T)
    bvb = np.ascontiguousarray(
        np.broadcast_to(bv.astype(ml_dtypes.bfloat16), (P, D))
    )

    in_maps = []
    for i in range(N_CORES):
        in_maps.append(
            {
                "xs": x[i * R : (i + 1) * R],
                "Wk": Wk,
                "Wv": Wv,
                "wq_sl": np.ascontiguousarray(Wq[i * SL : (i + 1) * SL]),
                "beta_col": beta_col,
                "bvb": bvb,
                "maskt": mask,
                "ident": identity,
                "ones1": ones_row,
            }
        )
    return in_maps


def kernel(x, Wq, bq, Wk, bk, Wv, bv):
    if "nc" not in _CACHED:
        _CACHED["nc"] = build_program()
    nc = _CACHED["nc"]
    in_maps = make_in_maps(x, Wq, bq, Wk, bk, Wv, bv)
    res = run_bass_kernel_spmd(nc, in_maps, list(range(N_CORES)))
    return np.concatenate([res.results[i]["out"] for i in range(N_CORES)], axis=0)
